# revision 19
# baseline (speedup 1.0000x reference)
"""Additive (Bahdanau) attention on 8 TRN2 NeuronCores.

Reference computation (T=2048, N=64, E=D=1024, A=256):
    e_proj = einsum('tne,ae->tna', enc_out, W_e)
    d_proj = einsum('nd,ad->na', dec_h, W_d)
    scores = einsum('tna,a->tn', tanh(e_proj + d_proj), v[0])
    alpha  = softmax(scores, axis=0)          # over T
    ctx    = einsum('tn,tne->ne', alpha, enc_out)
    returns (ctx, alpha)

Sharding: data-parallel over batch N (8 batches per core), weights
replicated. No collectives. Softmax uses the safe bound B = sum(|v|)
(|score| <= B since |tanh| <= 1), so exp(s - B) never overflows and no
max pass is needed: alpha = exp(s-B) / sum_t exp(s-B) == softmax(s).

Per-core layout choices:
  - enc shard is relaid out host-side to [n_local, E, T] so the e_proj
    matmul's moving operand ([e_chunk=128, t]) is contiguous.
  - e_proj on PE: lhsT = W_e^T chunk [e,a] (stationary), rhs = enc^T
    tile [e, t], out psum [a_chunk=128, t=512], accumulated over 8
    e-chunks. float32r moving path: 1 cycle/row (fp32 is 4).
  - tanh(e_proj + d_proj): one ACT op per psum tile; the d_proj column
    is the per-partition bias.
  - scores: PE matvec, lhsT = v^T chunk [a,1], accumulate 2 a-chunks.
  - exp with fused accum_out gives the denominator partials.
  - ctx: DVE tensor_tensor_reduce: (enc_tile * alpha_bcast) summed over
    the free (t) dim into one column of the ctx output tile. alpha is
    broadcast to 128 partitions via a K=1 PE matmul with a ones row.
"""

from contextlib import ExitStack

import numpy as np

import concourse.bacc as bacc
import concourse.tile as tile
from concourse import mybir
from concourse.bass_utils import run_bass_kernel_spmd

T, N, E, D, A = 2048, 64, 1024, 1024, 256
N_CORES = 8
NL = N // N_CORES          # batches per core
ECH = E // 128             # 8 e-chunks
ACH = A // 128             # 2 a-chunks
DCH = D // 128             # 8 d-chunks
TCH = 4                    # t-chunks per batch
TC = T // TCH              # 512

F32 = mybir.dt.float32
F32R = mybir.dt.float32r

# PE matmul dtype for the big matmuls: float32r streams at 1 cycle/row
# (plain fp32 takes 4). Set to False to fall back to full fp32.
import os
USE_F32R = os.environ.get("K_F32R", "1") == "1"
K_VARIANT = os.environ.get("K_VARIANT", "full")  # full|nottr|nobcast|noexp


def _mm_cast(ap):
    return ap.bitcast(F32R) if USE_F32R else ap


def build_nc(NL=NL):
    nc = bacc.Bacc(None)

    enc = nc.declare_dram_parameter("enc", [NL, E, T], F32, isOutput=False)
    WeT = nc.declare_dram_parameter("WeT", [E, A], F32, isOutput=False)
    dprojT = nc.declare_dram_parameter("dprojT", [A, NL], F32, isOutput=False)
    vT = nc.declare_dram_parameter("vT", [A, 1], F32, isOutput=False)
    negb = nc.declare_dram_parameter("negb", [1, 1], F32, isOutput=False)
    onesd = nc.declare_dram_parameter("ones", [1, 128], F32, isOutput=False)
    ctxT = nc.declare_dram_parameter("ctxT", [128, NL * ECH], F32, isOutput=True)
    alphaT = nc.declare_dram_parameter("alphaT", [NL, T], F32, isOutput=True)

    with tile.TileContext(nc) as tc, ExitStack() as ctx:
        singles = ctx.enter_context(tc.tile_pool(name="singles", bufs=1))
        enc_pool = ctx.enter_context(tc.tile_pool(name="encp", bufs=8))
        tanh_pool = ctx.enter_context(tc.tile_pool(name="tanhp", bufs=4))
        bcast_pool = ctx.enter_context(tc.tile_pool(name="bcastp", bufs=2))
        row_pool = ctx.enter_context(tc.tile_pool(name="rowp", bufs=2))
        ps_e = ctx.enter_context(tc.tile_pool(name="pse", bufs=3, space="PSUM"))
        ps_s = ctx.enter_context(tc.tile_pool(name="pss", bufs=2, space="PSUM"))
        ps_b = ctx.enter_context(tc.tile_pool(name="psb", bufs=2, space="PSUM"))
        dram_pool = ctx.enter_context(tc.tile_pool(name="dramp", bufs=2, space="DRAM"))
        scratch_pool = ctx.enter_context(tc.tile_pool(name="scratchp", bufs=1))
        alpha_pool = ctx.enter_context(tc.tile_pool(name="alphap", bufs=1))

        # ---- constants / weights ----
        we_all = singles.tile([128, ECH, A], F32, tag="weall")
        nc.sync.dma_start(
            out=_mm_cast(we_all[:]),
            in_=_mm_cast(WeT.rearrange("(c p) a -> p c a", p=128)))
        dproj_all = singles.tile([128, ACH, NL], F32, tag="dprojall")
        nc.sync.dma_start(
            out=dproj_all[:],
            in_=dprojT.rearrange("(c p) n -> p c n", p=128))
        v_all = singles.tile([128, ACH, 1], F32, tag="vall")
        nc.sync.dma_start(
            out=_mm_cast(v_all[:]),
            in_=_mm_cast(vT.rearrange("(c p) o -> p c o", p=128)))
        negb_sb = singles.tile([1, 1], F32, tag="negb")
        nc.sync.dma_start(out=negb_sb[:], in_=negb[:, :])
        ones_row = singles.tile([1, 128], F32, tag="ones")
        nc.sync.dma_start(out=ones_row[:].bitcast(F32R), in_=onesd[:, :].bitcast(F32R))
        ctx_sb = singles.tile([128, NL * ECH], F32, tag="ctxsb")

        # ---- main loop over local batches ----
        for n in range(NL):
            # enc^T tiles for this batch: 4 DMAs of [128, 2, T] (2 MiB each)
            et = []
            for pair in range(ECH // 2):
                t_ = enc_pool.tile([128, 2, T], F32, tag="enc")
                nc.sync.dma_start(
                    out=_mm_cast(t_[:]),
                    in_=_mm_cast(enc[n, 2 * pair * 128:(2 * pair + 2) * 128, :].rearrange(
                        "(c p) t -> p c t", p=128
                    )),
                )
                et.append(t_)

            def enc_tile(ec):
                return et[ec // 2][:, ec % 2, :]

            exp_n = row_pool.tile([1, T], F32, tag="exp")
            dparts = row_pool.tile([1, TCH], F32, tag="dparts")

            for tcid in range(TCH):
                t0 = tcid * TC
                tanh_t = []
                for ac in range(ACH):
                    ps = ps_e.tile([128, TC], F32, tag="pse")
                    for ec in range(ECH):
                        nc.tensor.matmul(
                            ps[:],
                            _mm_cast(we_all[:, ec, ac * 128:(ac + 1) * 128]),
                            _mm_cast(enc_tile(ec)[:, t0:t0 + TC]),
                            start=(ec == 0),
                            stop=(ec == ECH - 1),
                        )
                    th = tanh_pool.tile([128, TC], F32, tag="tanh")
                    nc.scalar.activation(
                        out=_mm_cast(th[:]),
                        in_=ps[:],
                        func=mybir.ActivationFunctionType.Tanh,
                        bias=dproj_all[:, ac, n:n + 1],
                        scale=1.0,
                    )
                    tanh_t.append(th)
                ps_sc = ps_s.tile([1, TC], F32, tag="pss")
                for ac in range(ACH):
                    nc.tensor.matmul(
                        ps_sc[:],
                        _mm_cast(v_all[:, ac, :]),
                        _mm_cast(tanh_t[ac][:]),
                        start=(ac == 0),
                        stop=(ac == ACH - 1),
                    )
                # exp(s - B) with fused partial-denominator accumulation
                nc.scalar.activation(
                    out=_mm_cast(exp_n[:, t0:t0 + TC]),
                    in_=ps_sc[:],
                    func=mybir.ActivationFunctionType.Exp,
                    bias=negb_sb[:],
                    scale=1.0,
                    accum_out=dparts[:, tcid:tcid + 1],
                )

            # denominator and its reciprocal
            denom = row_pool.tile([1, 1], F32, tag="denom")
            nc.vector.tensor_reduce(
                out=denom[:], in_=dparts[:], axis=mybir.AxisListType.X,
                op=mybir.AluOpType.add,
            )
            inv = row_pool.tile([1, 1], F32, tag="inv")
            nc.vector.reciprocal(inv[:], denom[:])

            # broadcast UNNORMALIZED exp to 128 partitions via K=1 matmul
            # (the bcast matmul operand's producer must be the f32r ACT write)
            ab = bcast_pool.tile([128, T], F32, tag="ab")
            for tcid in range(TCH if K_VARIANT in ("full", "nottr") else 0):
                t0 = tcid * TC
                ps = ps_b.tile([128, TC], F32, tag="psb")
                nc.tensor.matmul(
                    ps[:],
                    _mm_cast(ones_row[:]),
                    _mm_cast(exp_n[:, t0:t0 + TC]),
                    start=True,
                    stop=True,
                )
                nc.scalar.copy(ab[:, t0:t0 + TC], ps[:])

            # alpha = exp_n * inv into a separate tile (the verifier requires
            # every writer of a f32r-matmul operand to produce f32r)
            alpha_row = alpha_pool.tile([1, T], F32, tag="alpha")
            nc.vector.tensor_scalar_mul(alpha_row[:], exp_n[:], inv[:])
            nc.sync.dma_start(out=alphaT[n:n + 1, :], in_=alpha_row[:])

            # unnormalized ctx^T column per e-chunk:
            # DVE multiply (enc * alpha_bcast), then ACT copy with fused
            # accum_out reduction over t. (tensor_tensor_reduce crashes the
            # device on this runtime, so the mul and reduce are split.)
            for ec in range(ECH if K_VARIANT == "full" else 0):
                prod = scratch_pool.tile([128, T], F32, tag="ttrs")
                nc.vector.tensor_mul(prod[:], enc_tile(ec), ab[:])
                nc.scalar.activation(
                    out=prod[:],
                    in_=prod[:],
                    func=mybir.ActivationFunctionType.Copy,
                    accum_out=ctx_sb[:, n * ECH + ec:n * ECH + ec + 1],
                )

            # normalize this batch's ctx columns: needs inv on 128 partitions
            # (SBUF APs cannot broadcast across partitions; bounce via DRAM)
            inv_d = dram_pool.tile([1, 1], F32, tag="invd")
            nc.gpsimd.dma_start(out=inv_d[:], in_=inv[:])
            inv_bc = row_pool.tile([128, 1], F32, tag="invbc")
            nc.gpsimd.dma_start(out=inv_bc[:], in_=inv_d[:].to_broadcast((128, 1)))
            nc.vector.tensor_scalar_mul(
                ctx_sb[:, n * ECH:(n + 1) * ECH],
                ctx_sb[:, n * ECH:(n + 1) * ECH],
                inv_bc[:],
            )

        nc.sync.dma_start(out=ctxT[:, :], in_=ctx_sb[:])

    nc.compile()
    return nc


_NC_CACHE = None
TRACE = False          # set by test.py to capture the neuron profile
LAST_RESULT = None     # BassKernelResults of the most recent run


def kernel(enc_out, dec_h, W_e, W_d, v):
    global _NC_CACHE
    enc_out = np.asarray(enc_out, dtype=np.float32)
    dec_h = np.asarray(dec_h, dtype=np.float32)
    W_e = np.asarray(W_e, dtype=np.float32)
    W_d = np.asarray(W_d, dtype=np.float32)
    v = np.asarray(v, dtype=np.float32)

    if _NC_CACHE is None:
        _NC_CACHE = build_nc()
    nc = _NC_CACHE

    WeT = np.ascontiguousarray(W_e.T)                     # [E, A]
    vT = np.ascontiguousarray(v.reshape(1, A).T)          # [A, 1]
    negb = np.array([[-np.abs(v).sum()]], dtype=np.float32)

    in_maps = []
    for c in range(N_CORES):
        n0 = c * NL
        enc_s = np.ascontiguousarray(
            enc_out[:, n0:n0 + NL, :].transpose(1, 2, 0))  # [NL, E, T]
        dprojT_s = np.ascontiguousarray(W_d @ dec_h[n0:n0 + NL, :].T)  # [A, NL]
        in_maps.append({
            "enc": enc_s, "dprojT": dprojT_s, "WeT": WeT,
            "vT": vT, "negb": negb,
            "ones": np.ones((1, 128), dtype=np.float32),
        })

    res = run_bass_kernel_spmd(nc, in_maps, list(range(N_CORES)), trace=TRACE)
    global LAST_RESULT
    LAST_RESULT = res

    ctx = np.empty((N, E), dtype=np.float32)
    alpha = np.empty((T, N), dtype=np.float32)
    for c in range(N_CORES):
        n0 = c * NL
        ctxT = res.results[c]["ctxT"]                     # [128, NL*ECH]
        # ctxT[p, n*ECH + ec] == ctx[n0+n, ec*128 + p]
        ctx[n0:n0 + NL, :] = (
            ctxT.reshape(128, NL, ECH).transpose(1, 2, 0).reshape(NL, E))
        alpha[:, n0:n0 + NL] = res.results[c]["alphaT"].T
    return ctx, alpha


# revision 20
# speedup vs baseline: 1.1736x; 1.1736x over previous
"""Additive (Bahdanau) attention on 8 TRN2 NeuronCores.

Reference computation (T=2048, N=64, E=D=1024, A=256):
    e_proj = einsum('tne,ae->tna', enc_out, W_e)
    d_proj = einsum('nd,ad->na', dec_h, W_d)
    scores = einsum('tna,a->tn', tanh(e_proj + d_proj), v[0])
    alpha  = softmax(scores, axis=0)          # over T
    ctx    = einsum('tn,tne->ne', alpha, enc_out)
    returns (ctx, alpha)

Sharding: data-parallel over batch N (8 batches per core), weights
replicated; no collectives. Softmax uses the safe bound B = sum(|v|)
(|score| <= B since |tanh| <= 1), so exp(s - B) never overflows and no
max pass is needed: alpha = exp(s-B) / sum_t exp(s-B) == softmax(s).

Per-core pipeline (per local batch n):
  - enc shard is relaid out host-side to [n_local, E, T] bf16 so the
    e_proj moving operand ([e_chunk=128, t]) is contiguous and HBM
    traffic is halved.
  - e_proj on PE (bf16, 1 cyc/row; fp32 is 4, f32r is 2 on real HW):
    lhsT = W_e^T chunk [e,a], rhs = enc^T tile [e,t], psum
    [a_chunk=128, t=512] accumulated over 8 e-chunks (fp32 psum).
  - tanh(e_proj + d_proj): one ACT op per psum tile; the host-computed
    d_proj column is the per-partition bias. Output bf16.
  - scores: PE matvec (bf16), lhsT = v^T chunk [a,1], 2 a-chunks.
  - exp(s - B) on ACT with fused accum_out denominator partials (f32).
  - alpha broadcast to 128 partitions via a K=1 f32r matmul with a
    ones row; psum f32 copied to bf16 SBUF by ACT.
  - ctx column per e-chunk: DVE tensor_tensor mul (bf16 2x) then DVE
    tensor_scalar(x*1+0) with fused accum_out reduction (bf16 4x).
    (tensor_tensor_reduce crashes this runtime; tensor_scalar+accum
    and activation+accum are verified working on HW.)
"""

import os
from contextlib import ExitStack

import numpy as np

import concourse.bacc as bacc
import concourse.tile as tile
from concourse import mybir
from concourse.bass_utils import run_bass_kernel_spmd

T, N, E, D, A = 2048, 64, 1024, 1024, 256
N_CORES = 8
NL = N // N_CORES          # batches per core
ECH = E // 128             # 8 e-chunks
ACH = A // 128             # 2 a-chunks
TCH = 4                    # t-chunks per batch
TC = T // TCH              # 512

F32 = mybir.dt.float32
F32R = mybir.dt.float32r
BF16 = mybir.dt.bfloat16

ENC_BUFS = int(os.environ.get("K_ENC_BUFS", "12"))


def build_nc(NL=NL):
    nc = bacc.Bacc(None)

    enc = nc.declare_dram_parameter("enc", [NL, E, T], BF16, isOutput=False)
    dprojT = nc.declare_dram_parameter("dprojT", [A, NL], F32, isOutput=False)
    WeT = nc.declare_dram_parameter("WeT", [E, A], BF16, isOutput=False)
    vT = nc.declare_dram_parameter("vT", [A, 1], BF16, isOutput=False)
    negb = nc.declare_dram_parameter("negb", [1, 1], F32, isOutput=False)
    onesd = nc.declare_dram_parameter("ones", [1, 128], F32, isOutput=False)
    ctxT = nc.declare_dram_parameter("ctxT", [128, NL * ECH], F32, isOutput=True)
    alphaT = nc.declare_dram_parameter("alphaT", [NL, T], F32, isOutput=True)

    with tile.TileContext(nc) as tc, ExitStack() as ctx:
        singles = ctx.enter_context(tc.tile_pool(name="singles", bufs=1))
        enc_pool = ctx.enter_context(tc.tile_pool(name="encp", bufs=ENC_BUFS))
        tanh_pool = ctx.enter_context(tc.tile_pool(name="tanhp", bufs=4))
        bcast_pool = ctx.enter_context(tc.tile_pool(name="bcastp", bufs=2))
        row_pool = ctx.enter_context(tc.tile_pool(name="rowp", bufs=2))
        alpha_pool = ctx.enter_context(tc.tile_pool(name="alphap", bufs=1))
        scratch_pool = ctx.enter_context(tc.tile_pool(name="scratchp", bufs=2))
        ps_e = ctx.enter_context(tc.tile_pool(name="pse", bufs=3, space="PSUM"))
        ps_s = ctx.enter_context(tc.tile_pool(name="pss", bufs=2, space="PSUM"))
        ps_b = ctx.enter_context(tc.tile_pool(name="psb", bufs=2, space="PSUM"))
        dram_pool = ctx.enter_context(
            tc.tile_pool(name="dramp", bufs=2, space="DRAM"))

        # ---- constants / weights ----
        we_all = singles.tile([128, ECH, A], BF16, tag="weall")
        nc.sync.dma_start(
            out=we_all[:],
            in_=WeT.rearrange("(c p) a -> p c a", p=128))
        dproj_all = singles.tile([128, ACH, NL], F32, tag="dprojall")
        nc.sync.dma_start(
            out=dproj_all[:],
            in_=dprojT.rearrange("(c p) n -> p c n", p=128))
        v_all = singles.tile([128, ACH, 1], BF16, tag="vall")
        nc.sync.dma_start(
            out=v_all[:],
            in_=vT.rearrange("(c p) o -> p c o", p=128))
        negb_sb = singles.tile([1, 1], F32, tag="negb")
        nc.sync.dma_start(out=negb_sb[:], in_=negb[:, :])
        ones_row = singles.tile([1, 128], F32, tag="ones")
        nc.sync.dma_start(out=ones_row[:].bitcast(F32R),
                          in_=onesd[:, :].bitcast(F32R))
        ctx_sb = singles.tile([128, NL * ECH], F32, tag="ctxsb")

        # ---- main loop over local batches ----
        for n in range(NL):
            # enc^T tiles for this batch: 4 DMAs of [128, 2, T] bf16 (1 MiB)
            et = []
            for pair in range(ECH // 2):
                t_ = enc_pool.tile([128, 2, T], BF16, tag="enc")
                nc.sync.dma_start(
                    out=t_[:],
                    in_=enc[n, 2 * pair * 128:(2 * pair + 2) * 128, :]
                    .rearrange("(c p) t -> p c t", p=128),
                )
                et.append(t_)

            def enc_tile(ec):
                return et[ec // 2][:, ec % 2, :]

            exp_n = row_pool.tile([1, T], F32, tag="exp")
            dparts = row_pool.tile([1, TCH], F32, tag="dparts")

            for tcid in range(TCH):
                t0 = tcid * TC
                tanh_t = []
                for ac in range(ACH):
                    ps = ps_e.tile([128, TC], F32, tag="pse")
                    for ec in range(ECH):
                        nc.tensor.matmul(
                            ps[:],
                            we_all[:, ec, ac * 128:(ac + 1) * 128],
                            enc_tile(ec)[:, t0:t0 + TC],
                            start=(ec == 0),
                            stop=(ec == ECH - 1),
                        )
                    th = tanh_pool.tile([128, TC], BF16, tag="tanh")
                    nc.scalar.activation(
                        out=th[:],
                        in_=ps[:],
                        func=mybir.ActivationFunctionType.Tanh,
                        bias=dproj_all[:, ac, n:n + 1],
                        scale=1.0,
                    )
                    tanh_t.append(th)
                ps_sc = ps_s.tile([1, TC], F32, tag="pss")
                for ac in range(ACH):
                    nc.tensor.matmul(
                        ps_sc[:],
                        v_all[:, ac, :],
                        tanh_t[ac][:],
                        start=(ac == 0),
                        stop=(ac == ACH - 1),
                    )
                # exp(s - B) with fused partial-denominator accumulation;
                # written through an f32r view so the f32r bcast matmul's
                # operand has an f32r producer (bits are plain fp32).
                nc.scalar.activation(
                    out=exp_n[:, t0:t0 + TC].bitcast(F32R),
                    in_=ps_sc[:],
                    func=mybir.ActivationFunctionType.Exp,
                    bias=negb_sb[:],
                    scale=1.0,
                    accum_out=dparts[:, tcid:tcid + 1],
                )

            # denominator and its reciprocal
            denom = row_pool.tile([1, 1], F32, tag="denom")
            nc.vector.tensor_reduce(
                out=denom[:], in_=dparts[:], axis=mybir.AxisListType.X,
                op=mybir.AluOpType.add,
            )
            inv = row_pool.tile([1, 1], F32, tag="inv")
            nc.vector.reciprocal(inv[:], denom[:])

            # broadcast UNNORMALIZED exp to 128 partitions via K=1 f32r
            # matmul with a ones row; psum f32 -> bf16 SBUF copy on ACT
            ab = bcast_pool.tile([128, T], BF16, tag="ab")
            for tcid in range(TCH):
                t0 = tcid * TC
                ps = ps_b.tile([128, TC], F32, tag="psb")
                nc.tensor.matmul(
                    ps[:],
                    ones_row[:].bitcast(F32R),
                    exp_n[:, t0:t0 + TC].bitcast(F32R),
                    start=True,
                    stop=True,
                )
                nc.scalar.copy(ab[:, t0:t0 + TC], ps[:])

            # alpha = exp_n * inv (after the bcast reads), DMA out
            alpha_row = alpha_pool.tile([1, T], F32, tag="alpha")
            nc.vector.tensor_scalar_mul(alpha_row[:], exp_n[:], inv[:])
            nc.sync.dma_start(out=alphaT[n:n + 1, :], in_=alpha_row[:])

            # unnormalized ctx^T column per e-chunk: DVE bf16 multiply,
            # then DVE tensor_scalar(x*1+0) with fused accum reduction
            for ec in range(ECH):
                prod = scratch_pool.tile([128, T], BF16, tag="prod")
                nc.vector.tensor_mul(prod[:], enc_tile(ec), ab[:])
                nc.vector.tensor_scalar(
                    out=prod[:],
                    in0=prod[:],
                    scalar1=1.0,
                    scalar2=0.0,
                    op0=mybir.AluOpType.mult,
                    op1=mybir.AluOpType.add,
                    accum_out=ctx_sb[:, n * ECH + ec:n * ECH + ec + 1],
                )

            # normalize this batch's ctx columns: inv broadcast to 128
            # partitions via a DRAM bounce (SBUF APs cannot broadcast
            # across partitions)
            inv_d = dram_pool.tile([1, 1], F32, tag="invd")
            nc.gpsimd.dma_start(out=inv_d[:], in_=inv[:])
            inv_bc = row_pool.tile([128, 1], F32, tag="invbc")
            nc.gpsimd.dma_start(out=inv_bc[:],
                                in_=inv_d[:].to_broadcast((128, 1)))
            nc.vector.tensor_scalar_mul(
                ctx_sb[:, n * ECH:(n + 1) * ECH],
                ctx_sb[:, n * ECH:(n + 1) * ECH],
                inv_bc[:],
            )

        nc.sync.dma_start(out=ctxT[:, :], in_=ctx_sb[:])

    nc.compile()
    return nc


_NC_CACHE = None
TRACE = False          # set by test.py to capture the neuron profile
LAST_RESULT = None     # BassKernelResults of the most recent run


def kernel(enc_out, dec_h, W_e, W_d, v):
    global _NC_CACHE, LAST_RESULT
    import ml_dtypes
    bf16 = ml_dtypes.bfloat16

    enc_out = np.asarray(enc_out, dtype=np.float32)
    dec_h = np.asarray(dec_h, dtype=np.float32)
    W_e = np.asarray(W_e, dtype=np.float32)
    W_d = np.asarray(W_d, dtype=np.float32)
    v = np.asarray(v, dtype=np.float32)

    if _NC_CACHE is None:
        _NC_CACHE = build_nc()
    nc = _NC_CACHE

    WeT = np.ascontiguousarray(W_e.T).astype(bf16)             # [E, A]
    vT = np.ascontiguousarray(v.reshape(1, A).T).astype(bf16)  # [A, 1]
    negb = np.array([[-np.abs(v).sum()]], dtype=np.float32)

    in_maps = []
    for c in range(N_CORES):
        n0 = c * NL
        enc_s = np.ascontiguousarray(
            enc_out[:, n0:n0 + NL, :].transpose(1, 2, 0)).astype(bf16)
        dprojT_s = np.ascontiguousarray(W_d @ dec_h[n0:n0 + NL, :].T)
        in_maps.append({
            "enc": enc_s, "dprojT": dprojT_s, "WeT": WeT,
            "vT": vT, "negb": negb,
            "ones": np.ones((1, 128), dtype=np.float32),
        })

    res = run_bass_kernel_spmd(nc, in_maps, list(range(N_CORES)), trace=TRACE)
    LAST_RESULT = res

    ctx = np.empty((N, E), dtype=np.float32)
    alpha = np.empty((T, N), dtype=np.float32)
    for c in range(N_CORES):
        n0 = c * NL
        ctxT = res.results[c]["ctxT"]                     # [128, NL*ECH]
        # ctxT[p, n*ECH + ec] == ctx[n0+n, ec*128 + p]
        ctx[n0:n0 + NL, :] = (
            ctxT.reshape(128, NL, ECH).transpose(1, 2, 0).reshape(NL, E))
        alpha[:, n0:n0 + NL] = res.results[c]["alphaT"].T
    return ctx, alpha


# revision 21
# speedup vs baseline: 1.6013x; 1.3644x over previous
"""Additive (Bahdanau) attention on 8 TRN2 NeuronCores.

Reference computation (T=2048, N=64, E=D=1024, A=256):
    e_proj = einsum('tne,ae->tna', enc_out, W_e)
    d_proj = einsum('nd,ad->na', dec_h, W_d)
    scores = einsum('tna,a->tn', tanh(e_proj + d_proj), v[0])
    alpha  = softmax(scores, axis=0)          # over T
    ctx    = einsum('tn,tne->ne', alpha, enc_out)
    returns (ctx, alpha)

Sharding: data-parallel over batch N (8 batches per core), weights
replicated; no collectives. Softmax uses the safe bound B = sum(|v|)
(|score| <= B since |tanh| <= 1), so exp(s - B) never overflows and no
max pass is needed: alpha = exp(s-B) / sum_t exp(s-B) == softmax(s).

Per-core pipeline (per local batch n):
  - enc shard is relaid out host-side to [n_local, E, T] bf16 so the
    e_proj moving operand ([e_chunk=128, t]) is contiguous and HBM
    traffic is halved.
  - e_proj on PE (bf16, 1 cyc/row; fp32 is 4, f32r is 2 on real HW):
    lhsT = W_e^T chunk [e,a], rhs = enc^T tile [e,t], psum
    [a_chunk=128, t=512] accumulated over 8 e-chunks (fp32 psum).
  - tanh(e_proj + d_proj): one ACT op per psum tile; the host-computed
    d_proj column is the per-partition bias. Output bf16.
  - scores: PE matvec (bf16), lhsT = v^T chunk [a,1], 2 a-chunks.
  - exp(s - B) on ACT with fused accum_out denominator partials (f32).
  - alpha broadcast to 128 partitions via a K=1 f32r matmul with a
    ones row; psum f32 copied to bf16 SBUF by ACT.
  - ctx column per e-chunk: DVE tensor_tensor mul (bf16 2x) then DVE
    tensor_scalar(x*1+0) with fused accum_out reduction (bf16 4x).
    (tensor_tensor_reduce crashes this runtime; tensor_scalar+accum
    and activation+accum are verified working on HW.)
"""

import os
from contextlib import ExitStack

import numpy as np

import concourse.bacc as bacc
import concourse.tile as tile
from concourse import mybir
from concourse.bass_utils import run_bass_kernel_spmd

T, N, E, D, A = 2048, 64, 1024, 1024, 256
N_CORES = 8
NL = N // N_CORES          # batches per core
ECH = E // 128             # 8 e-chunks
ACH = A // 128             # 2 a-chunks
TCH = 4                    # t-chunks per batch
TC = T // TCH              # 512

F32 = mybir.dt.float32
F32R = mybir.dt.float32r
BF16 = mybir.dt.bfloat16

ENC_BUFS = int(os.environ.get("K_ENC_BUFS", "12"))
# how many of the 8 per-batch ctx reductions run as DVE-mul + ACT-accum
# instead of a single fused DVE scalar_tensor_tensor
ACT_REDUCES = int(os.environ.get("K_ACT_REDUCES", "0"))


def build_nc(NL=NL):
    nc = bacc.Bacc(None)

    enc = nc.declare_dram_parameter("enc", [NL, E, T], BF16, isOutput=False)
    dprojT = nc.declare_dram_parameter("dprojT", [A, NL], F32, isOutput=False)
    WeT = nc.declare_dram_parameter("WeT", [E, A], BF16, isOutput=False)
    vT = nc.declare_dram_parameter("vT", [A, 1], BF16, isOutput=False)
    negb = nc.declare_dram_parameter("negb", [1, 1], F32, isOutput=False)
    onesd = nc.declare_dram_parameter("ones", [1, 128], F32, isOutput=False)
    ctxT = nc.declare_dram_parameter("ctxT", [128, NL * ECH], F32, isOutput=True)
    alphaT = nc.declare_dram_parameter("alphaT", [NL, T], F32, isOutput=True)

    with tile.TileContext(nc) as tc, ExitStack() as ctx:
        singles = ctx.enter_context(tc.tile_pool(name="singles", bufs=1))
        enc_pool = ctx.enter_context(tc.tile_pool(name="encp", bufs=ENC_BUFS))
        tanh_pool = ctx.enter_context(tc.tile_pool(name="tanhp", bufs=4))
        bcast_pool = ctx.enter_context(tc.tile_pool(name="bcastp", bufs=2))
        row_pool = ctx.enter_context(tc.tile_pool(name="rowp", bufs=2))
        alpha_pool = ctx.enter_context(tc.tile_pool(name="alphap", bufs=1))
        scratch_pool = ctx.enter_context(tc.tile_pool(name="scratchp", bufs=2))
        ps_e = ctx.enter_context(tc.tile_pool(name="pse", bufs=3, space="PSUM"))
        ps_s = ctx.enter_context(tc.tile_pool(name="pss", bufs=2, space="PSUM"))
        ps_b = ctx.enter_context(tc.tile_pool(name="psb", bufs=2, space="PSUM"))
        dram_pool = ctx.enter_context(
            tc.tile_pool(name="dramp", bufs=2, space="DRAM"))

        # ---- constants / weights ----
        we_all = singles.tile([128, ECH, A], BF16, tag="weall")
        nc.sync.dma_start(
            out=we_all[:],
            in_=WeT.rearrange("(c p) a -> p c a", p=128))
        dproj_all = singles.tile([128, ACH, NL], F32, tag="dprojall")
        nc.sync.dma_start(
            out=dproj_all[:],
            in_=dprojT.rearrange("(c p) n -> p c n", p=128))
        v_all = singles.tile([128, ACH, 1], BF16, tag="vall")
        nc.sync.dma_start(
            out=v_all[:],
            in_=vT.rearrange("(c p) o -> p c o", p=128))
        negb_sb = singles.tile([1, 1], F32, tag="negb")
        nc.sync.dma_start(out=negb_sb[:], in_=negb[:, :])
        ones_row = singles.tile([1, 128], F32, tag="ones")
        nc.sync.dma_start(out=ones_row[:].bitcast(F32R),
                          in_=onesd[:, :].bitcast(F32R))
        ctx_sb = singles.tile([128, NL * ECH], F32, tag="ctxsb")

        # ---- main loop over local batches ----
        for n in range(NL):
            # enc^T tiles for this batch: 4 DMAs of [128, 2, T] bf16 (1 MiB)
            et = []
            for pair in range(ECH // 2):
                t_ = enc_pool.tile([128, 2, T], BF16, tag="enc")
                nc.sync.dma_start(
                    out=t_[:],
                    in_=enc[n, 2 * pair * 128:(2 * pair + 2) * 128, :]
                    .rearrange("(c p) t -> p c t", p=128),
                )
                et.append(t_)

            def enc_tile(ec):
                return et[ec // 2][:, ec % 2, :]

            exp_n = row_pool.tile([1, T], F32, tag="exp")
            dparts = row_pool.tile([1, TCH], F32, tag="dparts")

            for tcid in range(TCH):
                t0 = tcid * TC
                tanh_t = []
                for ac in range(ACH):
                    ps = ps_e.tile([128, TC], F32, tag="pse")
                    for ec in range(ECH):
                        nc.tensor.matmul(
                            ps[:],
                            we_all[:, ec, ac * 128:(ac + 1) * 128],
                            enc_tile(ec)[:, t0:t0 + TC],
                            start=(ec == 0),
                            stop=(ec == ECH - 1),
                        )
                    th = tanh_pool.tile([128, TC], BF16, tag="tanh")
                    nc.scalar.activation(
                        out=th[:],
                        in_=ps[:],
                        func=mybir.ActivationFunctionType.Tanh,
                        bias=dproj_all[:, ac, n:n + 1],
                        scale=1.0,
                    )
                    tanh_t.append(th)
                ps_sc = ps_s.tile([1, TC], F32, tag="pss")
                for ac in range(ACH):
                    nc.tensor.matmul(
                        ps_sc[:],
                        v_all[:, ac, :],
                        tanh_t[ac][:],
                        start=(ac == 0),
                        stop=(ac == ACH - 1),
                    )
                # exp(s - B) with fused partial-denominator accumulation;
                # written through an f32r view so the f32r bcast matmul's
                # operand has an f32r producer (bits are plain fp32).
                nc.scalar.activation(
                    out=exp_n[:, t0:t0 + TC].bitcast(F32R),
                    in_=ps_sc[:],
                    func=mybir.ActivationFunctionType.Exp,
                    bias=negb_sb[:],
                    scale=1.0,
                    accum_out=dparts[:, tcid:tcid + 1],
                )

            # denominator and its reciprocal
            denom = row_pool.tile([1, 1], F32, tag="denom")
            nc.vector.tensor_reduce(
                out=denom[:], in_=dparts[:], axis=mybir.AxisListType.X,
                op=mybir.AluOpType.add,
            )
            inv = row_pool.tile([1, 1], F32, tag="inv")
            nc.vector.reciprocal(inv[:], denom[:])

            # broadcast UNNORMALIZED exp to 128 partitions via K=1 f32r
            # matmul with a ones row; psum f32 -> bf16 SBUF copy on ACT
            ab = bcast_pool.tile([128, T], BF16, tag="ab")
            for tcid in range(TCH):
                t0 = tcid * TC
                ps = ps_b.tile([128, TC], F32, tag="psb")
                nc.tensor.matmul(
                    ps[:],
                    ones_row[:].bitcast(F32R),
                    exp_n[:, t0:t0 + TC].bitcast(F32R),
                    start=True,
                    stop=True,
                )
                nc.scalar.copy(ab[:, t0:t0 + TC], ps[:])

            # alpha = exp_n * inv (after the bcast reads), DMA out
            alpha_row = alpha_pool.tile([1, T], F32, tag="alpha")
            nc.vector.tensor_scalar_mul(alpha_row[:], exp_n[:], inv[:])
            nc.sync.dma_start(out=alphaT[n:n + 1, :], in_=alpha_row[:])

            # unnormalized ctx^T column per e-chunk: one fused DVE
            # scalar_tensor_tensor (out = (enc*1)*ab, accum_out = sum);
            # optionally the first ACT_REDUCES chunks instead do a DVE
            # multiply + ACT copy-with-accum to balance engine load.
            for ec in range(ECH):
                prod = scratch_pool.tile([128, T], BF16, tag="prod")
                col = ctx_sb[:, n * ECH + ec:n * ECH + ec + 1]
                if ec < ACT_REDUCES:
                    nc.vector.tensor_mul(prod[:], enc_tile(ec), ab[:])
                    nc.scalar.activation(
                        out=prod[:],
                        in_=prod[:],
                        func=mybir.ActivationFunctionType.Copy,
                        accum_out=col,
                    )
                else:
                    nc.vector.scalar_tensor_tensor(
                        out=prod[:],
                        in0=enc_tile(ec),
                        scalar=1.0,
                        in1=ab[:],
                        op0=mybir.AluOpType.mult,
                        op1=mybir.AluOpType.mult,
                        accum_out=col,
                    )

            # normalize this batch's ctx columns: inv broadcast to 128
            # partitions via a DRAM bounce (SBUF APs cannot broadcast
            # across partitions)
            inv_d = dram_pool.tile([1, 1], F32, tag="invd")
            nc.gpsimd.dma_start(out=inv_d[:], in_=inv[:])
            inv_bc = row_pool.tile([128, 1], F32, tag="invbc")
            nc.gpsimd.dma_start(out=inv_bc[:],
                                in_=inv_d[:].to_broadcast((128, 1)))
            nc.vector.tensor_scalar_mul(
                ctx_sb[:, n * ECH:(n + 1) * ECH],
                ctx_sb[:, n * ECH:(n + 1) * ECH],
                inv_bc[:],
            )

        nc.sync.dma_start(out=ctxT[:, :], in_=ctx_sb[:])

    nc.compile()
    return nc


_NC_CACHE = None
TRACE = False          # set by test.py to capture the neuron profile
LAST_RESULT = None     # BassKernelResults of the most recent run


def kernel(enc_out, dec_h, W_e, W_d, v):
    global _NC_CACHE, LAST_RESULT
    import ml_dtypes
    bf16 = ml_dtypes.bfloat16

    enc_out = np.asarray(enc_out, dtype=np.float32)
    dec_h = np.asarray(dec_h, dtype=np.float32)
    W_e = np.asarray(W_e, dtype=np.float32)
    W_d = np.asarray(W_d, dtype=np.float32)
    v = np.asarray(v, dtype=np.float32)

    if _NC_CACHE is None:
        _NC_CACHE = build_nc()
    nc = _NC_CACHE

    WeT = np.ascontiguousarray(W_e.T).astype(bf16)             # [E, A]
    vT = np.ascontiguousarray(v.reshape(1, A).T).astype(bf16)  # [A, 1]
    negb = np.array([[-np.abs(v).sum()]], dtype=np.float32)

    in_maps = []
    for c in range(N_CORES):
        n0 = c * NL
        enc_s = np.ascontiguousarray(
            enc_out[:, n0:n0 + NL, :].transpose(1, 2, 0)).astype(bf16)
        dprojT_s = np.ascontiguousarray(W_d @ dec_h[n0:n0 + NL, :].T)
        in_maps.append({
            "enc": enc_s, "dprojT": dprojT_s, "WeT": WeT,
            "vT": vT, "negb": negb,
            "ones": np.ones((1, 128), dtype=np.float32),
        })

    res = run_bass_kernel_spmd(nc, in_maps, list(range(N_CORES)), trace=TRACE)
    LAST_RESULT = res

    ctx = np.empty((N, E), dtype=np.float32)
    alpha = np.empty((T, N), dtype=np.float32)
    for c in range(N_CORES):
        n0 = c * NL
        ctxT = res.results[c]["ctxT"]                     # [128, NL*ECH]
        # ctxT[p, n*ECH + ec] == ctx[n0+n, ec*128 + p]
        ctx[n0:n0 + NL, :] = (
            ctxT.reshape(128, NL, ECH).transpose(1, 2, 0).reshape(NL, E))
        alpha[:, n0:n0 + NL] = res.results[c]["alphaT"].T
    return ctx, alpha


# revision 22
# speedup vs baseline: 1.6031x; 1.0011x over previous
"""Additive (Bahdanau) attention on 8 TRN2 NeuronCores.

Reference computation (T=2048, N=64, E=D=1024, A=256):
    e_proj = einsum('tne,ae->tna', enc_out, W_e)
    d_proj = einsum('nd,ad->na', dec_h, W_d)
    scores = einsum('tna,a->tn', tanh(e_proj + d_proj), v[0])
    alpha  = softmax(scores, axis=0)          # over T
    ctx    = einsum('tn,tne->ne', alpha, enc_out)
    returns (ctx, alpha)

Sharding: data-parallel over batch N (8 batches per core), weights
replicated; no collectives. Softmax uses the safe bound B = sum(|v|)
(|score| <= B since |tanh| <= 1), so exp(s - B) never overflows and no
max pass is needed: alpha = exp(s-B) / sum_t exp(s-B) == softmax(s).

Per-core pipeline (per local batch n):
  - enc shard is relaid out host-side to [n_local, E, T] bf16 so the
    e_proj moving operand ([e_chunk=128, t]) is contiguous and HBM
    traffic is halved.
  - e_proj on PE (bf16, 1 cyc/row; fp32 is 4, f32r is 2 on real HW):
    lhsT = W_e^T chunk [e,a], rhs = enc^T tile [e,t], psum
    [a_chunk=128, t=512] accumulated over 8 e-chunks (fp32 psum).
  - tanh(e_proj + d_proj): one ACT op per psum tile; the host-computed
    d_proj column is the per-partition bias. Output bf16.
  - scores: PE matvec (bf16), lhsT = v^T chunk [a,1], 2 a-chunks.
  - exp(s - B) on ACT with fused accum_out denominator partials (f32).
  - alpha broadcast to 128 partitions via a K=1 f32r matmul with a
    ones row; psum f32 copied to bf16 SBUF by ACT.
  - ctx column per e-chunk: DVE tensor_tensor mul (bf16 2x) then DVE
    tensor_scalar(x*1+0) with fused accum_out reduction (bf16 4x).
    (tensor_tensor_reduce crashes this runtime; tensor_scalar+accum
    and activation+accum are verified working on HW.)
"""

import os
from contextlib import ExitStack

import numpy as np

import concourse.bacc as bacc
import concourse.tile as tile
from concourse import mybir
from concourse.bass_utils import run_bass_kernel_spmd

T, N, E, D, A = 2048, 64, 1024, 1024, 256
N_CORES = 8
NL = N // N_CORES          # batches per core
ECH = E // 128             # 8 e-chunks
ACH = A // 128             # 2 a-chunks
TCH = 4                    # t-chunks per batch
TC = T // TCH              # 512

F32 = mybir.dt.float32
F32R = mybir.dt.float32r
BF16 = mybir.dt.bfloat16

ENC_BUFS = int(os.environ.get("K_ENC_BUFS", "12"))
# how many of the 8 per-batch ctx reductions run as DVE-mul + ACT-accum
# instead of a single fused DVE scalar_tensor_tensor
ACT_REDUCES = int(os.environ.get("K_ACT_REDUCES", "3"))


def build_nc(NL=NL):
    nc = bacc.Bacc(None)

    enc = nc.declare_dram_parameter("enc", [NL, E, T], BF16, isOutput=False)
    dprojT = nc.declare_dram_parameter("dprojT", [A, NL], F32, isOutput=False)
    WeT = nc.declare_dram_parameter("WeT", [E, A], BF16, isOutput=False)
    vT = nc.declare_dram_parameter("vT", [A, 1], BF16, isOutput=False)
    negb = nc.declare_dram_parameter("negb", [1, 1], F32, isOutput=False)
    onesd = nc.declare_dram_parameter("ones", [1, 128], F32, isOutput=False)
    ctxT = nc.declare_dram_parameter("ctxT", [128, NL * ECH], F32, isOutput=True)
    alphaT = nc.declare_dram_parameter("alphaT", [NL, T], F32, isOutput=True)

    with tile.TileContext(nc) as tc, ExitStack() as ctx:
        singles = ctx.enter_context(tc.tile_pool(name="singles", bufs=1))
        enc_pool = ctx.enter_context(tc.tile_pool(name="encp", bufs=ENC_BUFS))
        tanh_pool = ctx.enter_context(tc.tile_pool(name="tanhp", bufs=4))
        bcast_pool = ctx.enter_context(tc.tile_pool(name="bcastp", bufs=2))
        row_pool = ctx.enter_context(tc.tile_pool(name="rowp", bufs=2))
        alpha_pool = ctx.enter_context(tc.tile_pool(name="alphap", bufs=1))
        scratch_pool = ctx.enter_context(tc.tile_pool(name="scratchp", bufs=2))
        ps_e = ctx.enter_context(tc.tile_pool(name="pse", bufs=3, space="PSUM"))
        ps_s = ctx.enter_context(tc.tile_pool(name="pss", bufs=2, space="PSUM"))
        ps_b = ctx.enter_context(tc.tile_pool(name="psb", bufs=2, space="PSUM"))
        dram_pool = ctx.enter_context(
            tc.tile_pool(name="dramp", bufs=2, space="DRAM"))

        # ---- constants / weights ----
        we_all = singles.tile([128, ECH, A], BF16, tag="weall")
        nc.sync.dma_start(
            out=we_all[:],
            in_=WeT.rearrange("(c p) a -> p c a", p=128))
        dproj_all = singles.tile([128, ACH, NL], F32, tag="dprojall")
        nc.sync.dma_start(
            out=dproj_all[:],
            in_=dprojT.rearrange("(c p) n -> p c n", p=128))
        v_all = singles.tile([128, ACH, 1], BF16, tag="vall")
        nc.sync.dma_start(
            out=v_all[:],
            in_=vT.rearrange("(c p) o -> p c o", p=128))
        negb_sb = singles.tile([1, 1], F32, tag="negb")
        nc.sync.dma_start(out=negb_sb[:], in_=negb[:, :])
        ones_row = singles.tile([1, 128], F32, tag="ones")
        nc.sync.dma_start(out=ones_row[:].bitcast(F32R),
                          in_=onesd[:, :].bitcast(F32R))
        ctx_sb = singles.tile([128, NL * ECH], F32, tag="ctxsb")

        # ---- main loop over local batches ----
        for n in range(NL):
            # enc^T tiles for this batch: 4 DMAs of [128, 2, T] bf16 (1 MiB)
            et = []
            for pair in range(ECH // 2):
                t_ = enc_pool.tile([128, 2, T], BF16, tag="enc")
                nc.sync.dma_start(
                    out=t_[:],
                    in_=enc[n, 2 * pair * 128:(2 * pair + 2) * 128, :]
                    .rearrange("(c p) t -> p c t", p=128),
                )
                et.append(t_)

            def enc_tile(ec):
                return et[ec // 2][:, ec % 2, :]

            exp_n = row_pool.tile([1, T], F32, tag="exp")
            dparts = row_pool.tile([1, TCH], F32, tag="dparts")

            for tcid in range(TCH):
                t0 = tcid * TC
                tanh_t = []
                for ac in range(ACH):
                    ps = ps_e.tile([128, TC], F32, tag="pse")
                    for ec in range(ECH):
                        nc.tensor.matmul(
                            ps[:],
                            we_all[:, ec, ac * 128:(ac + 1) * 128],
                            enc_tile(ec)[:, t0:t0 + TC],
                            start=(ec == 0),
                            stop=(ec == ECH - 1),
                        )
                    th = tanh_pool.tile([128, TC], BF16, tag="tanh")
                    nc.scalar.activation(
                        out=th[:],
                        in_=ps[:],
                        func=mybir.ActivationFunctionType.Tanh,
                        bias=dproj_all[:, ac, n:n + 1],
                        scale=1.0,
                    )
                    tanh_t.append(th)
                ps_sc = ps_s.tile([1, TC], F32, tag="pss")
                for ac in range(ACH):
                    nc.tensor.matmul(
                        ps_sc[:],
                        v_all[:, ac, :],
                        tanh_t[ac][:],
                        start=(ac == 0),
                        stop=(ac == ACH - 1),
                    )
                # exp(s - B) with fused partial-denominator accumulation;
                # written through an f32r view so the f32r bcast matmul's
                # operand has an f32r producer (bits are plain fp32).
                nc.scalar.activation(
                    out=exp_n[:, t0:t0 + TC].bitcast(F32R),
                    in_=ps_sc[:],
                    func=mybir.ActivationFunctionType.Exp,
                    bias=negb_sb[:],
                    scale=1.0,
                    accum_out=dparts[:, tcid:tcid + 1],
                )

            # denominator and its reciprocal
            denom = row_pool.tile([1, 1], F32, tag="denom")
            nc.vector.tensor_reduce(
                out=denom[:], in_=dparts[:], axis=mybir.AxisListType.X,
                op=mybir.AluOpType.add,
            )
            inv = row_pool.tile([1, 1], F32, tag="inv")
            nc.vector.reciprocal(inv[:], denom[:])

            # broadcast UNNORMALIZED exp to 128 partitions via K=1 f32r
            # matmul with a ones row; psum f32 -> bf16 SBUF copy on ACT
            ab = bcast_pool.tile([128, T], BF16, tag="ab")
            for tcid in range(TCH):
                t0 = tcid * TC
                ps = ps_b.tile([128, TC], F32, tag="psb")
                nc.tensor.matmul(
                    ps[:],
                    ones_row[:].bitcast(F32R),
                    exp_n[:, t0:t0 + TC].bitcast(F32R),
                    start=True,
                    stop=True,
                )
                nc.scalar.copy(ab[:, t0:t0 + TC], ps[:])

            # alpha = exp_n * inv (after the bcast reads), DMA out
            alpha_row = alpha_pool.tile([1, T], F32, tag="alpha")
            nc.vector.tensor_scalar_mul(alpha_row[:], exp_n[:], inv[:])
            nc.gpsimd.dma_start(out=alphaT[n:n + 1, :], in_=alpha_row[:])

            # unnormalized ctx^T column per e-chunk: one fused DVE
            # scalar_tensor_tensor (out = (enc*1)*ab, accum_out = sum);
            # optionally the first ACT_REDUCES chunks instead do a DVE
            # multiply + ACT copy-with-accum to balance engine load.
            for ec in range(ECH):
                prod = scratch_pool.tile([128, T], BF16, tag="prod")
                col = ctx_sb[:, n * ECH + ec:n * ECH + ec + 1]
                if ec < ACT_REDUCES:
                    nc.vector.tensor_mul(prod[:], enc_tile(ec), ab[:])
                    nc.scalar.activation(
                        out=prod[:],
                        in_=prod[:],
                        func=mybir.ActivationFunctionType.Copy,
                        accum_out=col,
                    )
                else:
                    nc.vector.scalar_tensor_tensor(
                        out=prod[:],
                        in0=enc_tile(ec),
                        scalar=1.0,
                        in1=ab[:],
                        op0=mybir.AluOpType.mult,
                        op1=mybir.AluOpType.mult,
                        accum_out=col,
                    )

            # normalize this batch's ctx columns: inv broadcast to 128
            # partitions via a DRAM bounce (SBUF APs cannot broadcast
            # across partitions)
            inv_d = dram_pool.tile([1, 1], F32, tag="invd")
            nc.gpsimd.dma_start(out=inv_d[:], in_=inv[:])
            inv_bc = row_pool.tile([128, 1], F32, tag="invbc")
            nc.gpsimd.dma_start(out=inv_bc[:],
                                in_=inv_d[:].to_broadcast((128, 1)))
            nc.vector.tensor_scalar_mul(
                ctx_sb[:, n * ECH:(n + 1) * ECH],
                ctx_sb[:, n * ECH:(n + 1) * ECH],
                inv_bc[:],
            )

        nc.gpsimd.dma_start(out=ctxT[:, :], in_=ctx_sb[:])

    nc.compile()
    return nc


_NC_CACHE = None
TRACE = False          # set by test.py to capture the neuron profile
LAST_RESULT = None     # BassKernelResults of the most recent run


def kernel(enc_out, dec_h, W_e, W_d, v):
    global _NC_CACHE, LAST_RESULT
    import ml_dtypes
    bf16 = ml_dtypes.bfloat16

    enc_out = np.asarray(enc_out, dtype=np.float32)
    dec_h = np.asarray(dec_h, dtype=np.float32)
    W_e = np.asarray(W_e, dtype=np.float32)
    W_d = np.asarray(W_d, dtype=np.float32)
    v = np.asarray(v, dtype=np.float32)

    if _NC_CACHE is None:
        _NC_CACHE = build_nc()
    nc = _NC_CACHE

    WeT = np.ascontiguousarray(W_e.T).astype(bf16)             # [E, A]
    vT = np.ascontiguousarray(v.reshape(1, A).T).astype(bf16)  # [A, 1]
    negb = np.array([[-np.abs(v).sum()]], dtype=np.float32)

    in_maps = []
    for c in range(N_CORES):
        n0 = c * NL
        enc_s = np.ascontiguousarray(
            enc_out[:, n0:n0 + NL, :].transpose(1, 2, 0)).astype(bf16)
        dprojT_s = np.ascontiguousarray(W_d @ dec_h[n0:n0 + NL, :].T)
        in_maps.append({
            "enc": enc_s, "dprojT": dprojT_s, "WeT": WeT,
            "vT": vT, "negb": negb,
            "ones": np.ones((1, 128), dtype=np.float32),
        })

    res = run_bass_kernel_spmd(nc, in_maps, list(range(N_CORES)), trace=TRACE)
    LAST_RESULT = res

    ctx = np.empty((N, E), dtype=np.float32)
    alpha = np.empty((T, N), dtype=np.float32)
    for c in range(N_CORES):
        n0 = c * NL
        ctxT = res.results[c]["ctxT"]                     # [128, NL*ECH]
        # ctxT[p, n*ECH + ec] == ctx[n0+n, ec*128 + p]
        ctx[n0:n0 + NL, :] = (
            ctxT.reshape(128, NL, ECH).transpose(1, 2, 0).reshape(NL, E))
        alpha[:, n0:n0 + NL] = res.results[c]["alphaT"].T
    return ctx, alpha


# revision 23
# speedup vs baseline: 1.6244x; 1.0133x over previous
"""Additive (Bahdanau) attention on 8 TRN2 NeuronCores.

Reference computation (T=2048, N=64, E=D=1024, A=256):
    e_proj = einsum('tne,ae->tna', enc_out, W_e)
    d_proj = einsum('nd,ad->na', dec_h, W_d)
    scores = einsum('tna,a->tn', tanh(e_proj + d_proj), v[0])
    alpha  = softmax(scores, axis=0)          # over T
    ctx    = einsum('tn,tne->ne', alpha, enc_out)
    returns (ctx, alpha)

Sharding: data-parallel over batch N (8 batches per core), weights
replicated; no collectives. Softmax uses the safe bound B = sum(|v|)
(|score| <= B since |tanh| <= 1), so exp(s - B) never overflows and no
max pass is needed: alpha = exp(s-B) / sum_t exp(s-B) == softmax(s).

Per-core pipeline (per local batch n):
  - enc shard is relaid out host-side to [n_local, E, T] bf16 so the
    e_proj moving operand ([e_chunk=128, t]) is contiguous and HBM
    traffic is halved.
  - e_proj on PE (bf16, 1 cyc/row; fp32 is 4, f32r is 2 on real HW):
    lhsT = W_e^T chunk [e,a], rhs = enc^T tile [e,t], psum
    [a_chunk=128, t=512] accumulated over 8 e-chunks (fp32 psum).
  - tanh(e_proj + d_proj): one ACT op per psum tile; the host-computed
    d_proj column is the per-partition bias. Output bf16.
  - scores: PE matvec (bf16), lhsT = v^T chunk [a,1], 2 a-chunks.
  - exp(s - B) on ACT with fused accum_out denominator partials (f32).
  - alpha broadcast to 128 partitions via a K=1 f32r matmul with a
    ones row; psum f32 copied to bf16 SBUF by ACT.
  - ctx column per e-chunk: DVE tensor_tensor mul (bf16 2x) then DVE
    tensor_scalar(x*1+0) with fused accum_out reduction (bf16 4x).
    (tensor_tensor_reduce crashes this runtime; tensor_scalar+accum
    and activation+accum are verified working on HW.)
"""

import os
from contextlib import ExitStack

import numpy as np

import concourse.bacc as bacc
import concourse.tile as tile
from concourse import mybir
from concourse.bass_utils import run_bass_kernel_spmd

T, N, E, D, A = 2048, 64, 1024, 1024, 256
N_CORES = 8
NL = N // N_CORES          # batches per core
ECH = E // 128             # 8 e-chunks
ACH = A // 128             # 2 a-chunks
TCH = 4                    # t-chunks per batch
TC = T // TCH              # 512

F32 = mybir.dt.float32
F32R = mybir.dt.float32r
BF16 = mybir.dt.bfloat16

ENC_BUFS = int(os.environ.get("K_ENC_BUFS", "12"))
# how many of the 8 per-batch ctx reductions run as DVE-mul + ACT-accum
# instead of a single fused DVE scalar_tensor_tensor
ACT_REDUCES = int(os.environ.get("K_ACT_REDUCES", "0"))
# chunks whose multiply runs on GPSIMD (reduce on ACT), freeing DVE
GPS_REDUCES = int(os.environ.get("K_GPS_REDUCES", "2"))


def build_nc(NL=NL):
    nc = bacc.Bacc(None)

    enc = nc.declare_dram_parameter("enc", [NL, E, T], BF16, isOutput=False)
    dprojT = nc.declare_dram_parameter("dprojT", [A, NL], F32, isOutput=False)
    WeT = nc.declare_dram_parameter("WeT", [E, A], BF16, isOutput=False)
    vT = nc.declare_dram_parameter("vT", [A, 1], BF16, isOutput=False)
    negb = nc.declare_dram_parameter("negb", [1, 1], F32, isOutput=False)
    onesd = nc.declare_dram_parameter("ones", [1, 128], F32, isOutput=False)
    ctxT = nc.declare_dram_parameter("ctxT", [128, NL * ECH], F32, isOutput=True)
    alphaT = nc.declare_dram_parameter("alphaT", [NL, T], F32, isOutput=True)

    with tile.TileContext(nc) as tc, ExitStack() as ctx:
        singles = ctx.enter_context(tc.tile_pool(name="singles", bufs=1))
        enc_pool = ctx.enter_context(tc.tile_pool(name="encp", bufs=ENC_BUFS))
        tanh_pool = ctx.enter_context(tc.tile_pool(name="tanhp", bufs=4))
        bcast_pool = ctx.enter_context(tc.tile_pool(name="bcastp", bufs=2))
        row_pool = ctx.enter_context(tc.tile_pool(name="rowp", bufs=2))
        alpha_pool = ctx.enter_context(tc.tile_pool(name="alphap", bufs=1))
        scratch_pool = ctx.enter_context(tc.tile_pool(name="scratchp", bufs=2))
        ps_e = ctx.enter_context(tc.tile_pool(name="pse", bufs=3, space="PSUM"))
        ps_s = ctx.enter_context(tc.tile_pool(name="pss", bufs=2, space="PSUM"))
        ps_b = ctx.enter_context(tc.tile_pool(name="psb", bufs=2, space="PSUM"))
        dram_pool = ctx.enter_context(
            tc.tile_pool(name="dramp", bufs=2, space="DRAM"))

        # ---- constants / weights ----
        we_all = singles.tile([128, ECH, A], BF16, tag="weall")
        nc.sync.dma_start(
            out=we_all[:],
            in_=WeT.rearrange("(c p) a -> p c a", p=128))
        dproj_all = singles.tile([128, ACH, NL], F32, tag="dprojall")
        nc.sync.dma_start(
            out=dproj_all[:],
            in_=dprojT.rearrange("(c p) n -> p c n", p=128))
        v_all = singles.tile([128, ACH, 1], BF16, tag="vall")
        nc.sync.dma_start(
            out=v_all[:],
            in_=vT.rearrange("(c p) o -> p c o", p=128))
        negb_sb = singles.tile([1, 1], F32, tag="negb")
        nc.sync.dma_start(out=negb_sb[:], in_=negb[:, :])
        ones_row = singles.tile([1, 128], F32, tag="ones")
        nc.sync.dma_start(out=ones_row[:].bitcast(F32R),
                          in_=onesd[:, :].bitcast(F32R))
        ctx_sb = singles.tile([128, NL * ECH], F32, tag="ctxsb")

        # ---- main loop over local batches ----
        for n in range(NL):
            # enc^T tiles for this batch: 4 DMAs of [128, 2, T] bf16 (1 MiB)
            et = []
            for pair in range(ECH // 2):
                t_ = enc_pool.tile([128, 2, T], BF16, tag="enc")
                nc.sync.dma_start(
                    out=t_[:],
                    in_=enc[n, 2 * pair * 128:(2 * pair + 2) * 128, :]
                    .rearrange("(c p) t -> p c t", p=128),
                )
                et.append(t_)

            def enc_tile(ec):
                return et[ec // 2][:, ec % 2, :]

            exp_n = row_pool.tile([1, T], F32, tag="exp")
            dparts = row_pool.tile([1, TCH], F32, tag="dparts")

            for tcid in range(TCH):
                t0 = tcid * TC
                tanh_t = []
                for ac in range(ACH):
                    ps = ps_e.tile([128, TC], F32, tag="pse")
                    for ec in range(ECH):
                        nc.tensor.matmul(
                            ps[:],
                            we_all[:, ec, ac * 128:(ac + 1) * 128],
                            enc_tile(ec)[:, t0:t0 + TC],
                            start=(ec == 0),
                            stop=(ec == ECH - 1),
                        )
                    th = tanh_pool.tile([128, TC], BF16, tag="tanh")
                    nc.scalar.activation(
                        out=th[:],
                        in_=ps[:],
                        func=mybir.ActivationFunctionType.Tanh,
                        bias=dproj_all[:, ac, n:n + 1],
                        scale=1.0,
                    )
                    tanh_t.append(th)
                ps_sc = ps_s.tile([1, TC], F32, tag="pss")
                for ac in range(ACH):
                    nc.tensor.matmul(
                        ps_sc[:],
                        v_all[:, ac, :],
                        tanh_t[ac][:],
                        start=(ac == 0),
                        stop=(ac == ACH - 1),
                    )
                # exp(s - B) with fused partial-denominator accumulation;
                # written through an f32r view so the f32r bcast matmul's
                # operand has an f32r producer (bits are plain fp32).
                nc.scalar.activation(
                    out=exp_n[:, t0:t0 + TC].bitcast(F32R),
                    in_=ps_sc[:],
                    func=mybir.ActivationFunctionType.Exp,
                    bias=negb_sb[:],
                    scale=1.0,
                    accum_out=dparts[:, tcid:tcid + 1],
                )

            # denominator and its reciprocal
            denom = row_pool.tile([1, 1], F32, tag="denom")
            nc.vector.tensor_reduce(
                out=denom[:], in_=dparts[:], axis=mybir.AxisListType.X,
                op=mybir.AluOpType.add,
            )
            inv = row_pool.tile([1, 1], F32, tag="inv")
            nc.vector.reciprocal(inv[:], denom[:])

            # broadcast UNNORMALIZED exp to 128 partitions via K=1 f32r
            # matmul with a ones row; psum f32 -> bf16 SBUF copy on ACT
            ab = bcast_pool.tile([128, T], BF16, tag="ab")
            for tcid in range(TCH):
                t0 = tcid * TC
                ps = ps_b.tile([128, TC], F32, tag="psb")
                nc.tensor.matmul(
                    ps[:],
                    ones_row[:].bitcast(F32R),
                    exp_n[:, t0:t0 + TC].bitcast(F32R),
                    start=True,
                    stop=True,
                )
                nc.scalar.copy(ab[:, t0:t0 + TC], ps[:])

            # alpha = exp_n * inv (after the bcast reads), DMA out
            alpha_row = alpha_pool.tile([1, T], F32, tag="alpha")
            nc.vector.tensor_scalar_mul(alpha_row[:], exp_n[:], inv[:])
            nc.gpsimd.dma_start(out=alphaT[n:n + 1, :], in_=alpha_row[:])

            # unnormalized ctx^T column per e-chunk: one fused DVE
            # scalar_tensor_tensor (out = (enc*1)*ab, accum_out = sum);
            # optionally the first ACT_REDUCES chunks instead do a DVE
            # multiply + ACT copy-with-accum to balance engine load.
            for ec in range(ECH):
                prod = scratch_pool.tile([128, T], BF16, tag="prod")
                col = ctx_sb[:, n * ECH + ec:n * ECH + ec + 1]
                if ec < GPS_REDUCES:
                    nc.gpsimd.tensor_mul(prod[:], enc_tile(ec), ab[:])
                    nc.scalar.activation(
                        out=prod[:],
                        in_=prod[:],
                        func=mybir.ActivationFunctionType.Copy,
                        accum_out=col,
                    )
                elif ec < GPS_REDUCES + ACT_REDUCES:
                    nc.vector.tensor_mul(prod[:], enc_tile(ec), ab[:])
                    nc.scalar.activation(
                        out=prod[:],
                        in_=prod[:],
                        func=mybir.ActivationFunctionType.Copy,
                        accum_out=col,
                    )
                else:
                    nc.vector.scalar_tensor_tensor(
                        out=prod[:],
                        in0=enc_tile(ec),
                        scalar=1.0,
                        in1=ab[:],
                        op0=mybir.AluOpType.mult,
                        op1=mybir.AluOpType.mult,
                        accum_out=col,
                    )

            # normalize this batch's ctx columns: inv broadcast to 128
            # partitions via a DRAM bounce (SBUF APs cannot broadcast
            # across partitions)
            inv_d = dram_pool.tile([1, 1], F32, tag="invd")
            nc.gpsimd.dma_start(out=inv_d[:], in_=inv[:])
            inv_bc = row_pool.tile([128, 1], F32, tag="invbc")
            nc.gpsimd.dma_start(out=inv_bc[:],
                                in_=inv_d[:].to_broadcast((128, 1)))
            nc.vector.tensor_scalar_mul(
                ctx_sb[:, n * ECH:(n + 1) * ECH],
                ctx_sb[:, n * ECH:(n + 1) * ECH],
                inv_bc[:],
            )

        nc.gpsimd.dma_start(out=ctxT[:, :], in_=ctx_sb[:])

    nc.compile()
    return nc


_NC_CACHE = None
TRACE = False          # set by test.py to capture the neuron profile
LAST_RESULT = None     # BassKernelResults of the most recent run


def kernel(enc_out, dec_h, W_e, W_d, v):
    global _NC_CACHE, LAST_RESULT
    import ml_dtypes
    bf16 = ml_dtypes.bfloat16

    enc_out = np.asarray(enc_out, dtype=np.float32)
    dec_h = np.asarray(dec_h, dtype=np.float32)
    W_e = np.asarray(W_e, dtype=np.float32)
    W_d = np.asarray(W_d, dtype=np.float32)
    v = np.asarray(v, dtype=np.float32)

    if _NC_CACHE is None:
        _NC_CACHE = build_nc()
    nc = _NC_CACHE

    WeT = np.ascontiguousarray(W_e.T).astype(bf16)             # [E, A]
    vT = np.ascontiguousarray(v.reshape(1, A).T).astype(bf16)  # [A, 1]
    negb = np.array([[-np.abs(v).sum()]], dtype=np.float32)

    in_maps = []
    for c in range(N_CORES):
        n0 = c * NL
        enc_s = np.ascontiguousarray(
            enc_out[:, n0:n0 + NL, :].transpose(1, 2, 0)).astype(bf16)
        dprojT_s = np.ascontiguousarray(W_d @ dec_h[n0:n0 + NL, :].T)
        in_maps.append({
            "enc": enc_s, "dprojT": dprojT_s, "WeT": WeT,
            "vT": vT, "negb": negb,
            "ones": np.ones((1, 128), dtype=np.float32),
        })

    res = run_bass_kernel_spmd(nc, in_maps, list(range(N_CORES)), trace=TRACE)
    LAST_RESULT = res

    ctx = np.empty((N, E), dtype=np.float32)
    alpha = np.empty((T, N), dtype=np.float32)
    for c in range(N_CORES):
        n0 = c * NL
        ctxT = res.results[c]["ctxT"]                     # [128, NL*ECH]
        # ctxT[p, n*ECH + ec] == ctx[n0+n, ec*128 + p]
        ctx[n0:n0 + NL, :] = (
            ctxT.reshape(128, NL, ECH).transpose(1, 2, 0).reshape(NL, E))
        alpha[:, n0:n0 + NL] = res.results[c]["alphaT"].T
    return ctx, alpha
